# revision 19
# baseline (speedup 1.0000x reference)
"""CrossAttention Trainium2 kernel (nn_CrossAttention_1683627180152), v3.

Sharding: 8 cores = batch(2) x query-chunks(4). Each core handles one batch's
full context (KV replicated across the 4 cores of that batch) and 1024 query
rows: LN -> Q proj -> KV proj -> flash-style attention -> out proj -> LN.

v3 vs v2 (425k ns): phase C software-pipeline deepened (PV lags S by 3
key-blocks so the exp latency of 1.1-1.3us never stalls the PE; st psum
bufs=3, pt bufs=5), denominator moved to the FIRST V column so the softmax
normalize happens inline per head-pair group straight from PSUM (reciprocal
-> stride-0 DMA broadcast -> tensor_mul) instead of the v2 end-of-phase
DMA-scatter + broadcast-matmul tail, all input DMAs (x, ctx resident in
SBUF, weights) issued up-front on the Pool queue (36ns/issue vs 607ns on
sync -> PE never waits on DMA and the HAM clock stays at 2.4GHz), and x
shipped bf16 (halves the startup DMA).

Math notes (same as v2):
  - q scale (1/8 total) and LN gain g_x folded into Wq on the host.
  - S computed transposed; softmax'd P^T feeds PV with keys on partitions.
  - No max-subtraction (logits ~N(0,1)); masked keys get additive -1e30 bias
    (ACT tiles) or a zero multiplier+bias (DVE tiles -> int16 0 = bf16 +0.0).
  - Ones-column appended to V makes PV also produce softmax denominators
    at psum partition 64 (engine partition bases must be 32-aligned).
  - Null kv token lives in a padded 33rd key block.
"""

import numpy as np
import ml_dtypes

import concourse.bass as bass
import concourse.bacc as bacc
import concourse.mybir as mybir
from concourse.tile import TileContext
from concourse.bass_utils import run_bass_kernel_spmd

F32 = mybir.dt.float32
BF16 = mybir.dt.bfloat16
I16 = mybir.dt.int16
AF = mybir.ActivationFunctionType
ALU = mybir.AluOpType

HEADS = 8
D = 64
DIM = 512
B = 2
N = 4096
M = 4096
NSH = N // 4            # query rows per core
KCS = 33                # key blocks of 128 (incl. pad block)
MP = KCS * 128          # padded key count
PAIRS = 4
EPS = 1e-5
MASK_NEG = -1e30
LAG = 3                 # PV trails S by LAG key blocks

# Schraudolph bf16 exp constants: bits16 = round(SCH_A * x + SCH_B)
SCH_A = 128.0 / np.log(2.0)          # 184.6650
SCH_B = 127.0 * 128.0 - 5.625        # centered: rel err in [-3.3%, +3.3%]

# fraction of exp tiles on ScalarE (rest on VectorE): ACT ~1113 ns vs DVE
# ~1343 ns per [128,1024] tile; DVE also carries the per-group normalize
# (reciprocal + muls), so ACT takes 4/7 of the exp tiles.
ACT_PAT = [True, False, True, False, True, False, True]


def _emit(nc):
    x_d = nc.dram_tensor("x_sh", [NSH, DIM], BF16, kind="ExternalInput")
    ctx_d = nc.dram_tensor("ctx", [M, DIM], BF16, kind="ExternalInput")
    maskb_d = nc.dram_tensor("maskb", [128, KCS], F32, kind="ExternalInput")
    arow_d = nc.dram_tensor("arow", [128, KCS], F32, kind="ExternalInput")
    brow_d = nc.dram_tensor("brow", [128, KCS], F32, kind="ExternalInput")
    wq_d = nc.dram_tensor("wq", [DIM, DIM], BF16, kind="ExternalInput")
    wkv_d = nc.dram_tensor("wkv", [DIM, 2 * DIM], BF16, kind="ExternalInput")
    wo_d = nc.dram_tensor("wo", [DIM, DIM], BF16, kind="ExternalInput")
    gout_d = nc.dram_tensor("gout", [1, DIM], F32, kind="ExternalInput")
    nullk2_d = nc.dram_tensor("nullk2", [128, 1], BF16, kind="ExternalInput")
    nullvrow_d = nc.dram_tensor("nullvrow", [1, HEADS * 65], BF16,
                                kind="ExternalInput")
    out_d = nc.dram_tensor("out_sh", [NSH, DIM], F32, kind="ExternalOutput")

    ident_d = nc.inline_tensor(np.eye(128, dtype=ml_dtypes.bfloat16),
                               name="ident_np")
    ones_d = nc.inline_tensor(np.ones((1, 64), dtype=ml_dtypes.bfloat16),
                              name="ones_np")

    with TileContext(nc) as tc:
        with tc.tile_pool(name="consts", bufs=1) as consts, \
             tc.tile_pool(name="xin", bufs=1) as xin, \
             tc.tile_pool(name="cin", bufs=1) as cin, \
             tc.tile_pool(name="wts", bufs=1) as wts, \
             tc.tile_pool(name="qTp", bufs=1) as qTp:

            # ------- prefetch: one batched DMA per tensor, queues split -----
            # sync queue: the tensors phase A needs first
            ident = consts.tile([128, 128], BF16)
            nc.sync.dma_start(out=ident, in_=ident_d[:, :])
            x_sb = xin.tile([128, 8, DIM], BF16)
            for c in range(4):
                nc.sync.dma_start(
                    out=x_sb[:, 2 * c:2 * (c + 1), :],
                    in_=x_d[256 * c:256 * (c + 1), :].rearrange(
                        "(t p) d -> p t d", p=128))
            wq_sb = wts.tile([128, 4, DIM], BF16)
            nc.sync.dma_start(out=wq_sb[:, :, :],
                              in_=wq_d[:, :].rearrange("(k p) d -> p k d", p=128))
            wkv_sb = wts.tile([128, 4, 2 * DIM], BF16)
            nc.sync.dma_start(out=wkv_sb[:, :, :],
                              in_=wkv_d[:, :].rearrange("(k p) d -> p k d", p=128))
            # scalar queue: small constants first, then ctx in 8 chunks
            # (keeping ctx off the sync queue so the 4MB transfer cannot
            # delay x/wq; measured first-MM-at-33us when ctx went first)
            ones_sb = consts.tile([1, 64], BF16)
            nc.scalar.dma_start(out=ones_sb, in_=ones_d[:, :])
            maskb_sb = consts.tile([128, KCS], F32)
            nc.scalar.dma_start(out=maskb_sb, in_=maskb_d[:, :])
            arow_sb = consts.tile([128, KCS], F32)
            nc.scalar.dma_start(out=arow_sb, in_=arow_d[:, :])
            brow_sb = consts.tile([128, KCS], F32)
            nc.scalar.dma_start(out=brow_sb, in_=brow_d[:, :])
            nullk2_sb = consts.tile([128, 1], BF16)
            nc.scalar.dma_start(out=nullk2_sb, in_=nullk2_d[:, :])
            ctx_sb = cin.tile([128, 32, DIM], BF16)
            for c in range(8):
                nc.scalar.dma_start(
                    out=ctx_sb[:, 4 * c:4 * (c + 1), :],
                    in_=ctx_d[512 * c:512 * (c + 1), :].rearrange(
                        "(t p) d -> p t d", p=128))
            wo_sb = wts.tile([128, 4, DIM], BF16)
            nc.gpsimd.dma_start(out=wo_sb[:, :, :],
                                in_=wo_d[:, :].rearrange("(k p) d -> p k d", p=128))
            gout_sb = consts.tile([128, DIM], F32)
            g_ap = gout_d[:, :]
            g_bcast = bass.AP(tensor=g_ap.tensor, offset=g_ap.offset,
                              ap=[[0, 128]] + list(g_ap.ap[1:]))
            nc.gpsimd.dma_start(out=gout_sb, in_=g_bcast)
            eps_sb = consts.tile([128, 1], F32)
            nc.vector.memset(eps_sb, EPS)
            negone_sb = consts.tile([128, 1], F32)
            nc.vector.memset(negone_sb, -1.0)

            qT_sb = qTp.tile([128, PAIRS, NSH], BF16)
            outT_sb = qT_sb  # normalized output overwrites consumed qT slices

            # ---------------- phase A: LN1 + Q proj (per 4-tile batch) ------
            with tc.tile_pool(name="xnTp", bufs=1) as xnTp, \
                 tc.tile_pool(name="stA", bufs=1) as stA, \
                 tc.tile_pool(name="stT", bufs=3) as stT, \
                 tc.tile_pool(name="xbp", bufs=2) as xbp, \
                 tc.tile_pool(name="tppsA", bufs=2, space="PSUM") as tppsA, \
                 tc.tile_pool(name="qpps", bufs=2, space="PSUM") as qpps:

                mvx = stA.tile([128, 8, 2], F32)
                rstdx = stA.tile([128, 8], F32)
                xnT_sb = xnTp.tile([128, 4, NSH], BF16)
                for q5 in range(2):
                    for t in range(4 * q5, 4 * q5 + 4):
                        st6 = stT.tile([128, 6], F32, name=f"stx{t}", tag="stx")
                        nc.vector.bn_stats(out=st6, in_=x_sb[:, t, :])
                        nc.vector.bn_aggr(out=mvx[:, t, :], in_=st6)
                    nc.scalar.activation(
                        out=rstdx[:, 4 * q5:4 * q5 + 4],
                        in_=mvx[:, 4 * q5:4 * q5 + 4, 1:2].rearrange(
                            "p a b -> p (a b)"),
                        func=AF.Sqrt, bias=eps_sb[:, 0:1], scale=1.0)
                    nc.vector.reciprocal(out=rstdx[:, 4 * q5:4 * q5 + 4],
                                         in_=rstdx[:, 4 * q5:4 * q5 + 4])
                    for t in range(4 * q5, 4 * q5 + 4):
                        xb = xbp.tile([128, DIM], BF16, name=f"xb{t}", tag="xb")
                        nc.vector.tensor_scalar(out=xb, in0=x_sb[:, t, :],
                                                scalar1=mvx[:, t, 0:1],
                                                scalar2=rstdx[:, t:t + 1],
                                                op0=ALU.subtract, op1=ALU.mult)
                        pst = tppsA.tile([128, 512], BF16, name=f"ptx{t}",
                                         tag="tpA")
                        for j in range(4):
                            nc.tensor.transpose(pst[:, 128 * j:128 * (j + 1)],
                                                xb[:, 128 * j:128 * (j + 1)],
                                                ident[:])
                        if t % 2:
                            nc.scalar.copy(xnT_sb[:, :, 128 * t:128 * (t + 1)],
                                           pst[:].rearrange(
                                               "p (j c) -> p j c", j=4))
                        else:
                            nc.vector.tensor_copy(
                                xnT_sb[:, :, 128 * t:128 * (t + 1)],
                                pst[:].rearrange("p (j c) -> p j c", j=4))
                    for p in range(PAIRS):
                        psq = qpps.tile([128, 512], F32, name=f"psq{p}_{q5}",
                                        tag="psq")
                        for kc in range(4):
                            nc.tensor.matmul(psq[:],
                                             wq_sb[:, kc, 128 * p:128 * (p + 1)],
                                             xnT_sb[:, kc, 512 * q5:512 * (q5 + 1)],
                                             start=(kc == 0), stop=(kc == 3))
                        if p % 2:
                            nc.scalar.copy(qT_sb[:, p, 512 * q5:512 * (q5 + 1)],
                                           psq[:])
                        else:
                            nc.vector.tensor_copy(
                                qT_sb[:, p, 512 * q5:512 * (q5 + 1)], psq[:])

            # ---------------- phase B: ctxT -> kT, v ----------------
            with tc.tile_pool(name="kTp", bufs=1) as kTp, \
                 tc.tile_pool(name="vp", bufs=1) as vp:
                kT_sb = kTp.tile([128, PAIRS, MP], BF16)
                v_sb = vp.tile([128, KCS, HEADS, 65], BF16)

                with tc.tile_pool(name="ctxT", bufs=2) as ctxTp, \
                     tc.tile_pool(name="tppsB", bufs=3, space="PSUM") as tppsB, \
                     tc.tile_pool(name="kvps", bufs=4, space="PSUM") as kvps:

                    # pad block (kc=32): null token at key 4096, zeros elsewhere
                    nc.gpsimd.memset(kT_sb[:, :, 4096:MP], 0.0)
                    for p in range(PAIRS):
                        nc.gpsimd.tensor_copy(kT_sb[:, p, 4096:4097],
                                              nullk2_sb[:, 0:1])
                    nc.gpsimd.memset(v_sb[:, 32, :, :], 0.0)
                    nc.sync.dma_start(
                        out=v_sb[0:1, 32, :, :],
                        in_=nullvrow_d[:, :].rearrange("a (h c) -> a h c", h=HEADS))
                    # ones column is LAST: denominators land at psum
                    # partition 64 (engine partition bases must be 32-aligned)
                    nc.gpsimd.memset(v_sb[:, 0:32, :, 64:65], 1.0)

                    for s in range(8):
                        slab = ctxTp.tile([128, 4, 512], BF16, name=f"slab{s}",
                                          tag="slab")
                        for r in range(4):
                            pst = tppsB.tile([128, 512], BF16, name=f"ptc{s}_{r}",
                                             tag="tpB")
                            for j in range(4):
                                nc.tensor.transpose(
                                    pst[:, 128 * j:128 * (j + 1)],
                                    ctx_sb[:, 4 * s + r, 128 * j:128 * (j + 1)],
                                    ident[:])
                            if r % 2:
                                nc.scalar.copy(
                                    slab[:, :, 128 * r:128 * (r + 1)],
                                    pst[:].rearrange("p (j c) -> p j c", j=4))
                            else:
                                nc.vector.tensor_copy(
                                    slab[:, :, 128 * r:128 * (r + 1)],
                                    pst[:].rearrange("p (j c) -> p j c", j=4))
                        for p in range(PAIRS):
                            psk = kvps.tile([128, 512], F32, name=f"psk{s}_{p}",
                                            tag="pskv")
                            for kc in range(4):
                                nc.tensor.matmul(psk[:],
                                                 wkv_sb[:, kc, 128 * p:128 * (p + 1)],
                                                 slab[:, kc, :],
                                                 start=(kc == 0), stop=(kc == 3))
                            if p % 2:
                                nc.scalar.copy(kT_sb[:, p, 512 * s:512 * (s + 1)],
                                               psk[:])
                            else:
                                nc.vector.tensor_copy(
                                    kT_sb[:, p, 512 * s:512 * (s + 1)], psk[:])
                        for r in range(4):
                            psv = kvps.tile([128, 512], F32, name=f"psv{s}_{r}",
                                            tag="pskv")
                            for kc in range(4):
                                nc.tensor.matmul(psv[:],
                                                 slab[:, kc, 128 * r:128 * (r + 1)],
                                                 wkv_sb[:, kc, DIM:2 * DIM],
                                                 start=(kc == 0), stop=(kc == 3))
                            if r % 2:
                                nc.scalar.copy(
                                    v_sb[:, 4 * s + r, :, 0:64],
                                    psv[:].rearrange("p (h c) -> p h c", h=HEADS))
                            else:
                                nc.vector.tensor_copy(
                                    v_sb[:, 4 * s + r, :, 0:64],
                                    psv[:].rearrange("p (h c) -> p h c", h=HEADS))

                # ---------------- phase C: attention ----------------
                with tc.tile_pool(name="cps", bufs=1, space="PSUM") as cps, \
                     tc.tile_pool(name="ptp", bufs=1) as ptp, \
                     tc.tile_pool(name="asbp", bufs=1) as asbp:

                    ecnt = 0
                    den16 = asbp.tile([16, 512], F32)
                    asbs = {}

                    for p in range(PAIRS):
                        hA, hB = 2 * p, 2 * p + 1
                        for q5 in range(2):
                            qsl = slice(512 * q5, 512 * (q5 + 1))
                            accA = cps.tile([65, 512], F32, name=f"accA{p}_{q5}",
                                            tag="accA", bufs=1)
                            accB = cps.tile([65, 512], F32, name=f"accB{p}_{q5}",
                                            tag="accB", bufs=1)
                            pts = [None] * KCS

                            def emit_s_exp(kc):
                                nonlocal ecnt
                                st = cps.tile([128, 1024], F32,
                                              name=f"st{p}_{q5}_{kc}", tag="st",
                                              bufs=3)
                                ksl = slice(128 * kc, 128 * (kc + 1))
                                nc.tensor.matmul(st[:, 0:512], kT_sb[0:64, p, ksl],
                                                 qT_sb[0:64, p, qsl],
                                                 start=True, stop=True)
                                nc.tensor.matmul(st[:, 512:1024],
                                                 kT_sb[64:128, p, ksl],
                                                 qT_sb[64:128, p, qsl],
                                                 start=True, stop=True,
                                                 tile_position=(64, 0))
                                # split every tile's exp across BOTH engines
                                # (ACT native Exp cols 0:576, DVE Schraudolph
                                # cols 576:1024): halves the S->exp->st-free
                                # latency that bounds the pipeline period,
                                # and both halves write identical bf16-prob
                                # bytes so PV is unchanged.
                                pti = ptp.tile([128, 1024], I16,
                                               name=f"pt{p}_{q5}_{kc}",
                                               tag="pt", bufs=LAG + 2)
                                nc.scalar.activation(
                                    out=pti[:, 0:576].bitcast(BF16),
                                    in_=st[:, 0:576],
                                    func=AF.Exp,
                                    bias=maskb_sb[:, kc:kc + 1], scale=1.0)
                                nc.vector.tensor_scalar(
                                    out=pti[:, 576:1024], in0=st[:, 576:1024],
                                    scalar1=arow_sb[:, kc:kc + 1],
                                    scalar2=brow_sb[:, kc:kc + 1],
                                    op0=ALU.mult, op1=ALU.add)
                                ecnt += 1
                                pts[kc] = pti.bitcast(BF16)

                            def emit_pv(kc):
                                pt = pts[kc]
                                nc.tensor.matmul(accA[:], v_sb[:, kc, hA, :],
                                                 pt[:, 0:512],
                                                 start=(kc == 0), stop=(kc == KCS - 1),
                                                 skip_group_check=True)
                                nc.tensor.matmul(accB[:], v_sb[:, kc, hB, :],
                                                 pt[:, 512:1024],
                                                 start=(kc == 0), stop=(kc == KCS - 1),
                                                 skip_group_check=True)
                                pts[kc] = None

                            for kc in range(KCS + LAG):
                                if kc < KCS:
                                    emit_s_exp(kc)
                                if kc >= LAG:
                                    emit_pv(kc - LAG)

                            # acc -> SBUF (frees the acc bank for the next
                            # group); den rows collect into den16 for ONE
                            # batched reciprocal at the end of the phase
                            # (1-partition DVE/Pool reciprocals measured
                            # 2.8-7us and stalled the pipeline every group).
                            for idx, acc in enumerate([accA, accB]):
                                row = 4 * p + 2 * q5 + idx
                                asb = asbp.tile([65, 512], F32,
                                                name=f"asb{p}_{q5}_{idx}",
                                                tag="asb", bufs=16)
                                nc.scalar.copy(asb, acc[:])
                                nc.gpsimd.dma_start(out=den16[row:row + 1, :],
                                                    in_=asb[64:65, :])
                                asbs[row] = asb

                    # batched denominators: one reciprocal for all 16 rows,
                    # then DMA-scatter back to partition 0 for the broadcast
                    # matmuls (moving operand must start at partition 0).
                    recs = asbp.tile([16, 512], F32)
                    nc.vector.reciprocal(recs, den16[:])
                    rec16b = asbp.tile([16, 512], BF16)
                    nc.vector.tensor_copy(rec16b, recs[:])
                    recb = asbp.tile([1, 16, 512], BF16)
                    nc.gpsimd.dma_start(out=recb[0:1, :, :], in_=rec16b[:, :])
                    for p in range(PAIRS):
                        for q5 in range(2):
                            qsl = slice(512 * q5, 512 * (q5 + 1))
                            for idx in range(2):
                                row = 4 * p + 2 * q5 + idx
                                bc = cps.tile([64, 512], F32,
                                              name=f"bcf{row}", tag="st",
                                              bufs=3)
                                nc.tensor.matmul(bc[:], ones_sb[0:1, 0:64],
                                                 recb[0:1, row, :],
                                                 start=True, stop=True)
                                nc.vector.tensor_mul(
                                    outT_sb[64 * idx:64 * (idx + 1), p, qsl],
                                    asbs[row][0:64, :], bc[:])

            # ---------------- phase D: out proj + LN2 ----------------
            with tc.tile_pool(name="finps", bufs=2, space="PSUM") as finps, \
                 tc.tile_pool(name="stD", bufs=1) as stD, \
                 tc.tile_pool(name="yp", bufs=1) as yp, \
                 tc.tile_pool(name="stT2", bufs=3) as stT2, \
                 tc.tile_pool(name="yo", bufs=3) as yo:
                mvo = stD.tile([128, 8, 2], F32)
                rstdo = stD.tile([128, 8], F32)
                y_sb = yp.tile([128, 8, DIM], F32)
                for t in range(8):
                    fin = finps.tile([128, 512], F32, name=f"fin{t}", tag="fin")
                    for p in range(PAIRS):
                        nc.tensor.matmul(fin[:], outT_sb[:, p, 128 * t:128 * (t + 1)],
                                         wo_sb[:, p, :],
                                         start=(p == 0), stop=(p == 3))
                    st6 = stT2.tile([128, 6], F32, name=f"sty{t}", tag="sty")
                    nc.vector.bn_stats(out=st6, in_=fin[:])
                    nc.vector.bn_aggr(out=mvo[:, t, :], in_=st6)
                    # y' = mean - fin = -(fin - mean) on ACT; sign fixed by
                    # negating rstd below
                    nc.scalar.activation(out=y_sb[:, t, :], in_=fin[:],
                                         func=AF.Identity, bias=mvo[:, t, 0:1],
                                         scale=-1.0)
                nc.scalar.activation(out=rstdo,
                                     in_=mvo[:, :, 1:2].rearrange("p a b -> p (a b)"),
                                     func=AF.Sqrt, bias=eps_sb[:, 0:1], scale=1.0)
                nc.vector.reciprocal(out=rstdo, in_=rstdo)
                nc.vector.tensor_scalar_mul(out=rstdo, in0=rstdo,
                                            scalar1=negone_sb[:, 0:1])
                for t in range(8):
                    yt = yo.tile([128, DIM], F32, name=f"yo{t}", tag="yo")
                    # yt = (y' * -rstd) * g  in one DVE pass
                    nc.vector.scalar_tensor_tensor(out=yt, in0=y_sb[:, t, :],
                                                   scalar=rstdo[:, t:t + 1],
                                                   in1=gout_sb,
                                                   op0=ALU.mult, op1=ALU.mult)
                    nc.gpsimd.dma_start(out=out_d[128 * t:128 * (t + 1), :], in_=yt)


_NC_CACHE = None


def _build():
    global _NC_CACHE
    if _NC_CACHE is None:
        nc = bacc.Bacc(None, target_bir_lowering=False)
        _emit(nc)
        nc.compile()
        _NC_CACHE = nc
    return _NC_CACHE


def make_in_maps(x, context, mask, g_x, null_kv, Wq, Wkv, Wo, g_out):
    x = np.asarray(x, dtype=np.float32)
    context = np.asarray(context, dtype=np.float32)
    mask = np.asarray(mask)
    g_x = np.asarray(g_x, dtype=np.float32)
    null_kv = np.asarray(null_kv, dtype=np.float32)
    Wq = np.asarray(Wq, dtype=np.float32)
    Wkv = np.asarray(Wkv, dtype=np.float32)
    Wo = np.asarray(Wo, dtype=np.float32)
    g_out = np.asarray(g_out, dtype=np.float32)

    # fold LN1 gain and the attention scale (1/8 total) into Wq
    wq = (g_x.astype(np.float64)[:, None] * Wq.astype(np.float64) * 0.125
          ).astype(ml_dtypes.bfloat16)
    wq = np.ascontiguousarray(wq)
    wkv16 = Wkv.astype(ml_dtypes.bfloat16)
    wo16 = Wo.astype(ml_dtypes.bfloat16)
    ctx16 = context.astype(ml_dtypes.bfloat16)
    x16 = x.astype(ml_dtypes.bfloat16)

    # ACT additive mask bias [b, 128, 33]; DVE Schraudolph mult/bias rows
    maskb = np.full((B, 128, KCS), MASK_NEG, dtype=np.float32)
    mb = np.where(mask, 0.0, MASK_NEG).astype(np.float32)      # [b, 4096]
    maskb[:, :, :32] = mb.reshape(B, 32, 128).transpose(0, 2, 1)
    maskb[:, 0, 32] = 0.0
    vis = maskb == 0.0
    arow = np.where(vis, np.float32(SCH_A), np.float32(0.0)).astype(np.float32)
    brow = np.where(vis, np.float32(SCH_B), np.float32(0.0)).astype(np.float32)

    nullk2 = np.concatenate([null_kv[0], null_kv[0]]).reshape(128, 1)
    nullk2 = np.ascontiguousarray(nullk2.astype(ml_dtypes.bfloat16))
    # v row layout per head: [null_v (64) | 1 (denominator ones)]
    nullvrow = np.zeros((1, HEADS * 65), dtype=np.float32)
    for h in range(HEADS):
        nullvrow[0, 65 * h:65 * h + 64] = null_kv[1]
        nullvrow[0, 65 * h + 64] = 1.0
    nullvrow = nullvrow.astype(ml_dtypes.bfloat16)

    in_maps = []
    for core in range(8):
        bi, ci = divmod(core, 4)
        in_maps.append({
            "x_sh": np.ascontiguousarray(x16[bi, NSH * ci:NSH * (ci + 1)]),
            "ctx": np.ascontiguousarray(ctx16[bi]),
            "maskb": np.ascontiguousarray(maskb[bi]),
            "arow": np.ascontiguousarray(arow[bi]),
            "brow": np.ascontiguousarray(brow[bi]),
            "wq": wq,
            "wkv": wkv16,
            "wo": wo16,
            "gout": np.ascontiguousarray(g_out.reshape(1, DIM)),
            "nullk2": nullk2,
            "nullvrow": nullvrow,
        })
    return in_maps


def kernel(x, context, mask, g_x, null_kv, Wq, Wkv, Wo, g_out):
    nc = _build()
    in_maps = make_in_maps(x, context, mask, g_x, null_kv, Wq, Wkv, Wo, g_out)
    res = run_bass_kernel_spmd(nc, in_maps, core_ids=list(range(8)))
    out = np.empty((B, N, DIM), dtype=np.float32)
    for core in range(8):
        bi, ci = divmod(core, 4)
        out[bi, NSH * ci:NSH * (ci + 1)] = res.results[core]["out_sh"]
    return out


# revision 20
# speedup vs baseline: 1.0034x; 1.0034x over previous
"""CrossAttention Trainium2 kernel (nn_CrossAttention_1683627180152), v3.

Sharding: 8 cores = batch(2) x query-chunks(4). Each core handles one batch's
full context (KV replicated across the 4 cores of that batch) and 1024 query
rows: LN -> Q proj -> KV proj -> flash-style attention -> out proj -> LN.

v3 vs v2 (425k ns): phase C software-pipeline deepened (PV lags S by 3
key-blocks so the exp latency of 1.1-1.3us never stalls the PE; st psum
bufs=3, pt bufs=5), denominator moved to the FIRST V column so the softmax
normalize happens inline per head-pair group straight from PSUM (reciprocal
-> stride-0 DMA broadcast -> tensor_mul) instead of the v2 end-of-phase
DMA-scatter + broadcast-matmul tail, all input DMAs (x, ctx resident in
SBUF, weights) issued up-front on the Pool queue (36ns/issue vs 607ns on
sync -> PE never waits on DMA and the HAM clock stays at 2.4GHz), and x
shipped bf16 (halves the startup DMA).

Math notes (same as v2):
  - q scale (1/8 total) and LN gain g_x folded into Wq on the host.
  - S computed transposed; softmax'd P^T feeds PV with keys on partitions.
  - No max-subtraction (logits ~N(0,1)); masked keys get additive -1e30 bias
    (ACT tiles) or a zero multiplier+bias (DVE tiles -> int16 0 = bf16 +0.0).
  - Ones-column appended to V makes PV also produce softmax denominators
    at psum partition 64 (engine partition bases must be 32-aligned).
  - Null kv token lives in a padded 33rd key block.
"""

import numpy as np
import ml_dtypes

import concourse.bass as bass
import concourse.bacc as bacc
import concourse.mybir as mybir
from concourse.tile import TileContext
from concourse.bass_utils import run_bass_kernel_spmd

F32 = mybir.dt.float32
BF16 = mybir.dt.bfloat16
I16 = mybir.dt.int16
AF = mybir.ActivationFunctionType
ALU = mybir.AluOpType

HEADS = 8
D = 64
DIM = 512
B = 2
N = 4096
M = 4096
NSH = N // 4            # query rows per core
KCS = 33                # key blocks of 128 (incl. pad block)
MP = KCS * 128          # padded key count
PAIRS = 4
EPS = 1e-5
MASK_NEG = -1e30
LAG = 3                 # PV trails S by LAG key blocks

# Schraudolph bf16 exp constants: bits16 = round(SCH_A * x + SCH_B)
SCH_A = 128.0 / np.log(2.0)          # 184.6650
SCH_B = 127.0 * 128.0 - 5.625        # centered: rel err in [-3.3%, +3.3%]

# fraction of exp tiles on ScalarE (rest on VectorE): ACT ~1113 ns vs DVE
# ~1343 ns per [128,1024] tile; DVE also carries the per-group normalize
# (reciprocal + muls), so ACT takes 4/7 of the exp tiles.
ACT_PAT = [True, False, True, False, True, False, True]


def _emit(nc):
    x_d = nc.dram_tensor("x_sh", [NSH, DIM], BF16, kind="ExternalInput")
    ctx_d = nc.dram_tensor("ctx", [M, DIM], BF16, kind="ExternalInput")
    maskb_d = nc.dram_tensor("maskb", [128, KCS], F32, kind="ExternalInput")
    arow_d = nc.dram_tensor("arow", [128, KCS], F32, kind="ExternalInput")
    brow_d = nc.dram_tensor("brow", [128, KCS], F32, kind="ExternalInput")
    wq_d = nc.dram_tensor("wq", [DIM, DIM], BF16, kind="ExternalInput")
    wkv_d = nc.dram_tensor("wkv", [DIM, 2 * DIM], BF16, kind="ExternalInput")
    wo_d = nc.dram_tensor("wo", [DIM, DIM], BF16, kind="ExternalInput")
    gout_d = nc.dram_tensor("gout", [1, DIM], F32, kind="ExternalInput")
    nullk2_d = nc.dram_tensor("nullk2", [128, 1], BF16, kind="ExternalInput")
    nullvrow_d = nc.dram_tensor("nullvrow", [1, HEADS * 65], BF16,
                                kind="ExternalInput")
    out_d = nc.dram_tensor("out_sh", [NSH, DIM], F32, kind="ExternalOutput")

    ident_d = nc.inline_tensor(np.eye(128, dtype=ml_dtypes.bfloat16),
                               name="ident_np")
    ones_d = nc.inline_tensor(np.ones((1, 64), dtype=ml_dtypes.bfloat16),
                              name="ones_np")

    with TileContext(nc) as tc:
        with tc.tile_pool(name="consts", bufs=1) as consts, \
             tc.tile_pool(name="xin", bufs=1) as xin, \
             tc.tile_pool(name="cin", bufs=1) as cin, \
             tc.tile_pool(name="wts", bufs=1) as wts, \
             tc.tile_pool(name="qTp", bufs=1) as qTp:

            # ------- prefetch: one batched DMA per tensor, queues split -----
            # sync queue: the tensors phase A needs first
            ident = consts.tile([128, 128], BF16)
            nc.sync.dma_start(out=ident, in_=ident_d[:, :])
            x_sb = xin.tile([128, 8, DIM], BF16)
            nc.sync.dma_start(out=x_sb[:, :, :],
                              in_=x_d[:, :].rearrange("(t p) d -> p t d", p=128))
            wq_sb = wts.tile([128, 4, DIM], BF16)
            nc.sync.dma_start(out=wq_sb[:, :, :],
                              in_=wq_d[:, :].rearrange("(k p) d -> p k d", p=128))
            wkv_sb = wts.tile([128, 4, 2 * DIM], BF16)
            nc.sync.dma_start(out=wkv_sb[:, :, :],
                              in_=wkv_d[:, :].rearrange("(k p) d -> p k d", p=128))
            # scalar queue: small constants first, then ctx in 8 chunks
            # (keeping ctx off the sync queue so the 4MB transfer cannot
            # delay x/wq; measured first-MM-at-33us when ctx went first)
            ones_sb = consts.tile([1, 64], BF16)
            nc.scalar.dma_start(out=ones_sb, in_=ones_d[:, :])
            maskb_sb = consts.tile([128, KCS], F32)
            nc.scalar.dma_start(out=maskb_sb, in_=maskb_d[:, :])
            arow_sb = consts.tile([128, KCS], F32)
            nc.scalar.dma_start(out=arow_sb, in_=arow_d[:, :])
            brow_sb = consts.tile([128, KCS], F32)
            nc.scalar.dma_start(out=brow_sb, in_=brow_d[:, :])
            nullk2_sb = consts.tile([128, 1], BF16)
            nc.scalar.dma_start(out=nullk2_sb, in_=nullk2_d[:, :])
            ctx_sb = cin.tile([128, 32, DIM], BF16)
            for c in range(8):
                nc.scalar.dma_start(
                    out=ctx_sb[:, 4 * c:4 * (c + 1), :],
                    in_=ctx_d[512 * c:512 * (c + 1), :].rearrange(
                        "(t p) d -> p t d", p=128))
            wo_sb = wts.tile([128, 4, DIM], BF16)
            nc.gpsimd.dma_start(out=wo_sb[:, :, :],
                                in_=wo_d[:, :].rearrange("(k p) d -> p k d", p=128))
            gout_sb = consts.tile([128, DIM], F32)
            g_ap = gout_d[:, :]
            g_bcast = bass.AP(tensor=g_ap.tensor, offset=g_ap.offset,
                              ap=[[0, 128]] + list(g_ap.ap[1:]))
            nc.gpsimd.dma_start(out=gout_sb, in_=g_bcast)
            eps_sb = consts.tile([128, 1], F32)
            nc.vector.memset(eps_sb, EPS)
            negone_sb = consts.tile([128, 1], F32)
            nc.vector.memset(negone_sb, -1.0)

            qT_sb = qTp.tile([128, PAIRS, NSH], BF16)
            outT_sb = qT_sb  # normalized output overwrites consumed qT slices

            # ---------------- phase A: LN1 + Q proj (per 4-tile batch) ------
            with tc.tile_pool(name="xnTp", bufs=1) as xnTp, \
                 tc.tile_pool(name="stA", bufs=1) as stA, \
                 tc.tile_pool(name="stT", bufs=3) as stT, \
                 tc.tile_pool(name="xbp", bufs=2) as xbp, \
                 tc.tile_pool(name="tppsA", bufs=2, space="PSUM") as tppsA, \
                 tc.tile_pool(name="qpps", bufs=2, space="PSUM") as qpps:

                mvx = stA.tile([128, 8, 2], F32)
                rstdx = stA.tile([128, 8], F32)
                xnT_sb = xnTp.tile([128, 4, NSH], BF16)
                for q5 in range(2):
                    for t in range(4 * q5, 4 * q5 + 4):
                        st6 = stT.tile([128, 6], F32, name=f"stx{t}", tag="stx")
                        nc.vector.bn_stats(out=st6, in_=x_sb[:, t, :])
                        nc.vector.bn_aggr(out=mvx[:, t, :], in_=st6)
                    nc.scalar.activation(
                        out=rstdx[:, 4 * q5:4 * q5 + 4],
                        in_=mvx[:, 4 * q5:4 * q5 + 4, 1:2].rearrange(
                            "p a b -> p (a b)"),
                        func=AF.Sqrt, bias=eps_sb[:, 0:1], scale=1.0)
                    nc.vector.reciprocal(out=rstdx[:, 4 * q5:4 * q5 + 4],
                                         in_=rstdx[:, 4 * q5:4 * q5 + 4])
                    for t in range(4 * q5, 4 * q5 + 4):
                        xb = xbp.tile([128, DIM], BF16, name=f"xb{t}", tag="xb")
                        nc.vector.tensor_scalar(out=xb, in0=x_sb[:, t, :],
                                                scalar1=mvx[:, t, 0:1],
                                                scalar2=rstdx[:, t:t + 1],
                                                op0=ALU.subtract, op1=ALU.mult)
                        pst = tppsA.tile([128, 512], BF16, name=f"ptx{t}",
                                         tag="tpA")
                        for j in range(4):
                            nc.tensor.transpose(pst[:, 128 * j:128 * (j + 1)],
                                                xb[:, 128 * j:128 * (j + 1)],
                                                ident[:])
                        if t % 2:
                            nc.scalar.copy(xnT_sb[:, :, 128 * t:128 * (t + 1)],
                                           pst[:].rearrange(
                                               "p (j c) -> p j c", j=4))
                        else:
                            nc.vector.tensor_copy(
                                xnT_sb[:, :, 128 * t:128 * (t + 1)],
                                pst[:].rearrange("p (j c) -> p j c", j=4))
                    for p in range(PAIRS):
                        psq = qpps.tile([128, 512], F32, name=f"psq{p}_{q5}",
                                        tag="psq")
                        for kc in range(4):
                            nc.tensor.matmul(psq[:],
                                             wq_sb[:, kc, 128 * p:128 * (p + 1)],
                                             xnT_sb[:, kc, 512 * q5:512 * (q5 + 1)],
                                             start=(kc == 0), stop=(kc == 3))
                        if p % 2:
                            nc.scalar.copy(qT_sb[:, p, 512 * q5:512 * (q5 + 1)],
                                           psq[:])
                        else:
                            nc.vector.tensor_copy(
                                qT_sb[:, p, 512 * q5:512 * (q5 + 1)], psq[:])

            # ---------------- phase B: ctxT -> kT, v ----------------
            with tc.tile_pool(name="kTp", bufs=1) as kTp, \
                 tc.tile_pool(name="vp", bufs=1) as vp:
                kT_sb = kTp.tile([128, PAIRS, MP], BF16)
                v_sb = vp.tile([128, KCS, HEADS, 65], BF16)

                with tc.tile_pool(name="ctxT", bufs=2) as ctxTp, \
                     tc.tile_pool(name="tppsB", bufs=3, space="PSUM") as tppsB, \
                     tc.tile_pool(name="kvps", bufs=4, space="PSUM") as kvps:

                    # pad block (kc=32): null token at key 4096, zeros elsewhere
                    nc.gpsimd.memset(kT_sb[:, :, 4096:MP], 0.0)
                    for p in range(PAIRS):
                        nc.gpsimd.tensor_copy(kT_sb[:, p, 4096:4097],
                                              nullk2_sb[:, 0:1])
                    nc.gpsimd.memset(v_sb[:, 32, :, :], 0.0)
                    nc.sync.dma_start(
                        out=v_sb[0:1, 32, :, :],
                        in_=nullvrow_d[:, :].rearrange("a (h c) -> a h c", h=HEADS))
                    # ones column is LAST: denominators land at psum
                    # partition 64 (engine partition bases must be 32-aligned)
                    nc.gpsimd.memset(v_sb[:, 0:32, :, 64:65], 1.0)

                    for s in range(8):
                        slab = ctxTp.tile([128, 4, 512], BF16, name=f"slab{s}",
                                          tag="slab")
                        for r in range(4):
                            pst = tppsB.tile([128, 512], BF16, name=f"ptc{s}_{r}",
                                             tag="tpB")
                            for j in range(4):
                                nc.tensor.transpose(
                                    pst[:, 128 * j:128 * (j + 1)],
                                    ctx_sb[:, 4 * s + r, 128 * j:128 * (j + 1)],
                                    ident[:])
                            if r % 2:
                                nc.scalar.copy(
                                    slab[:, :, 128 * r:128 * (r + 1)],
                                    pst[:].rearrange("p (j c) -> p j c", j=4))
                            else:
                                nc.vector.tensor_copy(
                                    slab[:, :, 128 * r:128 * (r + 1)],
                                    pst[:].rearrange("p (j c) -> p j c", j=4))
                        for p in range(PAIRS):
                            psk = kvps.tile([128, 512], F32, name=f"psk{s}_{p}",
                                            tag="pskv")
                            for kc in range(4):
                                nc.tensor.matmul(psk[:],
                                                 wkv_sb[:, kc, 128 * p:128 * (p + 1)],
                                                 slab[:, kc, :],
                                                 start=(kc == 0), stop=(kc == 3))
                            if p % 2:
                                nc.scalar.copy(kT_sb[:, p, 512 * s:512 * (s + 1)],
                                               psk[:])
                            else:
                                nc.vector.tensor_copy(
                                    kT_sb[:, p, 512 * s:512 * (s + 1)], psk[:])
                        for r in range(4):
                            psv = kvps.tile([128, 512], F32, name=f"psv{s}_{r}",
                                            tag="pskv")
                            for kc in range(4):
                                nc.tensor.matmul(psv[:],
                                                 slab[:, kc, 128 * r:128 * (r + 1)],
                                                 wkv_sb[:, kc, DIM:2 * DIM],
                                                 start=(kc == 0), stop=(kc == 3))
                            if r % 2:
                                nc.scalar.copy(
                                    v_sb[:, 4 * s + r, :, 0:64],
                                    psv[:].rearrange("p (h c) -> p h c", h=HEADS))
                            else:
                                nc.vector.tensor_copy(
                                    v_sb[:, 4 * s + r, :, 0:64],
                                    psv[:].rearrange("p (h c) -> p h c", h=HEADS))

                # ---------------- phase C: attention ----------------
                with tc.tile_pool(name="cps", bufs=1, space="PSUM") as cps, \
                     tc.tile_pool(name="ptp", bufs=1) as ptp, \
                     tc.tile_pool(name="asbp", bufs=1) as asbp:

                    ecnt = 0
                    den16 = asbp.tile([16, 512], F32)
                    asbs = {}

                    for p in range(PAIRS):
                        hA, hB = 2 * p, 2 * p + 1
                        for q5 in range(2):
                            qsl = slice(512 * q5, 512 * (q5 + 1))
                            accA = cps.tile([65, 512], F32, name=f"accA{p}_{q5}",
                                            tag="accA", bufs=1)
                            accB = cps.tile([65, 512], F32, name=f"accB{p}_{q5}",
                                            tag="accB", bufs=1)
                            pts = [None] * KCS

                            def emit_s_exp(kc):
                                nonlocal ecnt
                                st = cps.tile([128, 1024], F32,
                                              name=f"st{p}_{q5}_{kc}", tag="st",
                                              bufs=3)
                                ksl = slice(128 * kc, 128 * (kc + 1))
                                nc.tensor.matmul(st[:, 0:512], kT_sb[0:64, p, ksl],
                                                 qT_sb[0:64, p, qsl],
                                                 start=True, stop=True)
                                nc.tensor.matmul(st[:, 512:1024],
                                                 kT_sb[64:128, p, ksl],
                                                 qT_sb[64:128, p, qsl],
                                                 start=True, stop=True,
                                                 tile_position=(64, 0))
                                if ACT_PAT[ecnt % 7]:
                                    pt = ptp.tile([128, 1024], BF16,
                                                  name=f"pa{p}_{q5}_{kc}",
                                                  tag="pt", bufs=LAG + 2)
                                    nc.scalar.activation(out=pt, in_=st[:],
                                                         func=AF.Exp,
                                                         bias=maskb_sb[:, kc:kc + 1],
                                                         scale=1.0)
                                else:
                                    pti = ptp.tile([128, 1024], I16,
                                                   name=f"pv{p}_{q5}_{kc}",
                                                   tag="pt", bufs=LAG + 2)
                                    nc.vector.tensor_scalar(
                                        out=pti, in0=st[:],
                                        scalar1=arow_sb[:, kc:kc + 1],
                                        scalar2=brow_sb[:, kc:kc + 1],
                                        op0=ALU.mult, op1=ALU.add)
                                    pt = pti.bitcast(BF16)
                                ecnt += 1
                                pts[kc] = pt

                            def emit_pv(kc):
                                pt = pts[kc]
                                nc.tensor.matmul(accA[:], v_sb[:, kc, hA, :],
                                                 pt[:, 0:512],
                                                 start=(kc == 0), stop=(kc == KCS - 1),
                                                 skip_group_check=True)
                                nc.tensor.matmul(accB[:], v_sb[:, kc, hB, :],
                                                 pt[:, 512:1024],
                                                 start=(kc == 0), stop=(kc == KCS - 1),
                                                 skip_group_check=True)
                                pts[kc] = None

                            for kc in range(KCS + LAG):
                                if kc < KCS:
                                    emit_s_exp(kc)
                                if kc >= LAG:
                                    emit_pv(kc - LAG)

                            # acc -> SBUF (frees the acc bank for the next
                            # group); den rows collect into den16 for ONE
                            # batched reciprocal at the end of the phase
                            # (1-partition DVE/Pool reciprocals measured
                            # 2.8-7us and stalled the pipeline every group).
                            for idx, acc in enumerate([accA, accB]):
                                row = 4 * p + 2 * q5 + idx
                                asb = asbp.tile([65, 512], F32,
                                                name=f"asb{p}_{q5}_{idx}",
                                                tag="asb", bufs=16)
                                nc.scalar.copy(asb, acc[:])
                                nc.gpsimd.dma_start(out=den16[row:row + 1, :],
                                                    in_=asb[64:65, :])
                                asbs[row] = asb

                    # batched denominators: one reciprocal for all 16 rows,
                    # then DMA-scatter back to partition 0 for the broadcast
                    # matmuls (moving operand must start at partition 0).
                    recs = asbp.tile([16, 512], F32)
                    nc.vector.reciprocal(recs, den16[:])
                    rec16b = asbp.tile([16, 512], BF16)
                    nc.vector.tensor_copy(rec16b, recs[:])
                    recb = asbp.tile([1, 16, 512], BF16)
                    nc.gpsimd.dma_start(out=recb[0:1, :, :], in_=rec16b[:, :])
                    for p in range(PAIRS):
                        for q5 in range(2):
                            qsl = slice(512 * q5, 512 * (q5 + 1))
                            for idx in range(2):
                                row = 4 * p + 2 * q5 + idx
                                bc = cps.tile([64, 512], F32,
                                              name=f"bcf{row}", tag="st",
                                              bufs=3)
                                nc.tensor.matmul(bc[:], ones_sb[0:1, 0:64],
                                                 recb[0:1, row, :],
                                                 start=True, stop=True)
                                nc.vector.tensor_mul(
                                    outT_sb[64 * idx:64 * (idx + 1), p, qsl],
                                    asbs[row][0:64, :], bc[:])

            # ---------------- phase D: out proj + LN2 ----------------
            with tc.tile_pool(name="finps", bufs=2, space="PSUM") as finps, \
                 tc.tile_pool(name="stD", bufs=1) as stD, \
                 tc.tile_pool(name="yp", bufs=1) as yp, \
                 tc.tile_pool(name="stT2", bufs=3) as stT2, \
                 tc.tile_pool(name="yo", bufs=3) as yo:
                mvo = stD.tile([128, 8, 2], F32)
                rstdo = stD.tile([128, 8], F32)
                y_sb = yp.tile([128, 8, DIM], F32)
                for t in range(8):
                    fin = finps.tile([128, 512], F32, name=f"fin{t}", tag="fin")
                    for p in range(PAIRS):
                        nc.tensor.matmul(fin[:], outT_sb[:, p, 128 * t:128 * (t + 1)],
                                         wo_sb[:, p, :],
                                         start=(p == 0), stop=(p == 3))
                    st6 = stT2.tile([128, 6], F32, name=f"sty{t}", tag="sty")
                    nc.vector.bn_stats(out=st6, in_=fin[:])
                    nc.vector.bn_aggr(out=mvo[:, t, :], in_=st6)
                    # y' = mean - fin = -(fin - mean) on ACT; sign fixed by
                    # negating rstd below
                    nc.scalar.activation(out=y_sb[:, t, :], in_=fin[:],
                                         func=AF.Identity, bias=mvo[:, t, 0:1],
                                         scale=-1.0)
                nc.scalar.activation(out=rstdo,
                                     in_=mvo[:, :, 1:2].rearrange("p a b -> p (a b)"),
                                     func=AF.Sqrt, bias=eps_sb[:, 0:1], scale=1.0)
                nc.vector.reciprocal(out=rstdo, in_=rstdo)
                nc.vector.tensor_scalar_mul(out=rstdo, in0=rstdo,
                                            scalar1=negone_sb[:, 0:1])
                for t in range(8):
                    yt = yo.tile([128, DIM], F32, name=f"yo{t}", tag="yo")
                    # yt = (y' * -rstd) * g  in one DVE pass
                    nc.vector.scalar_tensor_tensor(out=yt, in0=y_sb[:, t, :],
                                                   scalar=rstdo[:, t:t + 1],
                                                   in1=gout_sb,
                                                   op0=ALU.mult, op1=ALU.mult)
                    nc.gpsimd.dma_start(out=out_d[128 * t:128 * (t + 1), :], in_=yt)


_NC_CACHE = None


def _build():
    global _NC_CACHE
    if _NC_CACHE is None:
        nc = bacc.Bacc(None, target_bir_lowering=False)
        _emit(nc)
        nc.compile()
        _NC_CACHE = nc
    return _NC_CACHE


def make_in_maps(x, context, mask, g_x, null_kv, Wq, Wkv, Wo, g_out):
    x = np.asarray(x, dtype=np.float32)
    context = np.asarray(context, dtype=np.float32)
    mask = np.asarray(mask)
    g_x = np.asarray(g_x, dtype=np.float32)
    null_kv = np.asarray(null_kv, dtype=np.float32)
    Wq = np.asarray(Wq, dtype=np.float32)
    Wkv = np.asarray(Wkv, dtype=np.float32)
    Wo = np.asarray(Wo, dtype=np.float32)
    g_out = np.asarray(g_out, dtype=np.float32)

    # fold LN1 gain and the attention scale (1/8 total) into Wq
    wq = (g_x.astype(np.float64)[:, None] * Wq.astype(np.float64) * 0.125
          ).astype(ml_dtypes.bfloat16)
    wq = np.ascontiguousarray(wq)
    wkv16 = Wkv.astype(ml_dtypes.bfloat16)
    wo16 = Wo.astype(ml_dtypes.bfloat16)
    ctx16 = context.astype(ml_dtypes.bfloat16)
    x16 = x.astype(ml_dtypes.bfloat16)

    # ACT additive mask bias [b, 128, 33]; DVE Schraudolph mult/bias rows
    maskb = np.full((B, 128, KCS), MASK_NEG, dtype=np.float32)
    mb = np.where(mask, 0.0, MASK_NEG).astype(np.float32)      # [b, 4096]
    maskb[:, :, :32] = mb.reshape(B, 32, 128).transpose(0, 2, 1)
    maskb[:, 0, 32] = 0.0
    vis = maskb == 0.0
    arow = np.where(vis, np.float32(SCH_A), np.float32(0.0)).astype(np.float32)
    brow = np.where(vis, np.float32(SCH_B), np.float32(0.0)).astype(np.float32)

    nullk2 = np.concatenate([null_kv[0], null_kv[0]]).reshape(128, 1)
    nullk2 = np.ascontiguousarray(nullk2.astype(ml_dtypes.bfloat16))
    # v row layout per head: [null_v (64) | 1 (denominator ones)]
    nullvrow = np.zeros((1, HEADS * 65), dtype=np.float32)
    for h in range(HEADS):
        nullvrow[0, 65 * h:65 * h + 64] = null_kv[1]
        nullvrow[0, 65 * h + 64] = 1.0
    nullvrow = nullvrow.astype(ml_dtypes.bfloat16)

    in_maps = []
    for core in range(8):
        bi, ci = divmod(core, 4)
        in_maps.append({
            "x_sh": np.ascontiguousarray(x16[bi, NSH * ci:NSH * (ci + 1)]),
            "ctx": np.ascontiguousarray(ctx16[bi]),
            "maskb": np.ascontiguousarray(maskb[bi]),
            "arow": np.ascontiguousarray(arow[bi]),
            "brow": np.ascontiguousarray(brow[bi]),
            "wq": wq,
            "wkv": wkv16,
            "wo": wo16,
            "gout": np.ascontiguousarray(g_out.reshape(1, DIM)),
            "nullk2": nullk2,
            "nullvrow": nullvrow,
        })
    return in_maps


def kernel(x, context, mask, g_x, null_kv, Wq, Wkv, Wo, g_out):
    nc = _build()
    in_maps = make_in_maps(x, context, mask, g_x, null_kv, Wq, Wkv, Wo, g_out)
    res = run_bass_kernel_spmd(nc, in_maps, core_ids=list(range(8)))
    out = np.empty((B, N, DIM), dtype=np.float32)
    for core in range(8):
        bi, ci = divmod(core, 4)
        out[bi, NSH * ci:NSH * (ci + 1)] = res.results[core]["out_sh"]
    return out
